# revision 19
# baseline (speedup 1.0000x reference)
"""GCN layer (SpMM + linear) on 8 Trainium2 NeuronCores — exact-packed dest-banded tiles, host-dense selection (bf16).

out[i] = (sum_{e: edge_row[e]==i} edge_val[e] * x[edge_col[e]]) @ W.T + b

Destination rows are partitioned across 8 cores (6250 each) into 13 PSUM
groups of 496 rows.  Per (group, source-half) bucket, edges are sorted by
destination and packed 128 per gather tile — no per-window padding.  Tile t's
destinations fall in a narrow data-derived band [db[t], db[t]+BW); the
selection matrix sval[slot, dest-db] (bf16, host-precomputed, resident in
SBUF) is dense over the band, so duplicate (src,dst) edges just sum.

Each slot gathers one 256B bf16 x row via SWDGE dma_gather (int16 indices,
lo/hi source halves on separate queues).  matmul(lhsT=gathered, rhs=sval
band) accumulates agg.T[feat, dest] into the group's PSUM bank.  Epilogue per
group: copy to SBUF, project with W.T (fp32), add bias, DMA out.
"""

import math
from dataclasses import dataclass

import numpy as np

GR = 496          # dest rows per PSUM group
CAP = 128         # slots per gather tile
D = 128           # feature dim


@dataclass(frozen=True)
class Cfg:
    n_nodes: int = 50000
    n_edges: int = 800000
    n_cores: int = 8
    chunk: int = 8            # tiles per dma_gather call
    gather_queues: int = 4
    gather_bufs: int = 8
    agg_bufs: int = 2         # PSUM agg groups in flight
    gelem: int = 128          # gather elems per descriptor (diagnostic only)
    dma_scratch: int = 16384  # SWDGE descriptor ring carveout bytes/partition
    skip_compute: bool = False
    loop_n: int = 0
    repeats: int = 1

    @property
    def split(self) -> int:
        return self.n_nodes // 2

    @property
    def rows_per_core(self) -> int:
        return self.n_nodes // self.n_cores

    @property
    def n_groups(self) -> int:
        return math.ceil(self.rows_per_core / GR)


@dataclass(frozen=True)
class Plan:
    tc: tuple    # tc[g][s]: tiles per (group, stream), max over cores
    db: tuple    # db[g][s][t]: band start column of tile t
    bw: int      # band width (compiled free size of the sval matmul)

    def t_stream(self, s):
        return sum(t[s] for t in self.tc)


def _preprocess(cfg: Cfg, x, edge_row, edge_col, edge_val, W, b):
    import ml_dtypes

    RPC = cfg.rows_per_core
    NG = cfg.n_groups
    SPLIT = cfg.split

    x = np.asarray(x)
    edge_row = np.asarray(edge_row)
    edge_col = np.asarray(edge_col)
    edge_val = np.asarray(edge_val)

    xb = np.ascontiguousarray(x.astype(ml_dtypes.bfloat16))
    xplo = np.ascontiguousarray(xb[:SPLIT])
    xphi = np.ascontiguousarray(xb[SPLIT:])

    # Pass 1: per (core, group, stream) dest-sorted edge arrays.
    per_core = []
    for c in range(cfg.n_cores):
        e0, e1 = np.searchsorted(edge_row, [c * RPC, (c + 1) * RPC])
        r_loc = edge_row[e0:e1] - c * RPC
        g_of_e = r_loc // GR
        d_loc = r_loc - g_of_e * GR
        src = edge_col[e0:e1].astype(np.int64)
        s_of_e = (src >= SPLIT).astype(np.int64)
        val = edge_val[e0:e1].astype(np.float64)
        buckets = {}
        for g in range(NG):
            for s in range(2):
                m = (g_of_e == g) & (s_of_e == s)
                order = np.argsort(d_loc[m], kind="stable")
                buckets[(g, s)] = (src[m][order] - s * SPLIT,
                                  d_loc[m][order], val[m][order])
        per_core.append(buckets)

    # Plan: tile counts, band starts, band width (shared across cores).
    tc = []
    for g in range(NG):
        tc.append(tuple(
            max(math.ceil(len(pc[(g, s)][0]) / CAP) for pc in per_core)
            for s in range(2)))

    db = []
    bw = 0
    for g in range(NG):
        row = []
        for s in range(2):
            nt = tc[g][s]
            starts = np.full(nt, GR, dtype=np.int64)
            ends = np.zeros(nt, dtype=np.int64)
            for pc in per_core:
                dd = pc[(g, s)][1]
                for t in range(nt):
                    seg = dd[t * CAP:(t + 1) * CAP]
                    if len(seg):
                        starts[t] = min(starts[t], seg[0])
                        ends[t] = max(ends[t], seg[-1] + 1)
            starts = np.minimum(starts, ends)  # empty tiles -> band at end
            bw = max(bw, int((ends - starts).max(initial=0)))
            row.append(tuple(int(v) for v in starts))
        db.append(tuple(row))
    bw = math.ceil(bw / 16) * 16
    # clamp band starts so db+bw stays inside the group
    db = tuple(
        tuple(tuple(min(v, GR - bw) for v in row_s) for row_s in row)
        for row in db)
    plan = Plan(tc=tuple(tc), db=db, bw=bw)

    CH = cfg.chunk
    tp = [math.ceil(plan.t_stream(s) / CH) * CH for s in range(2)]

    wt = np.asarray(W).T.astype(np.float32)
    bb = np.tile(np.asarray(b)[None, :].astype(np.float32), (128, 1))

    def wrap_idx(idx_lin):
        return np.tile(np.ascontiguousarray(idx_lin.reshape(-1, 16).T), (8, 1))

    in_maps = []
    for c in range(cfg.n_cores):
        buckets = per_core[c]
        idx_lin = [np.zeros(tp[s] * CAP, dtype=np.int16) for s in range(2)]
        sval = [np.zeros((128, tp[s] * bw), dtype=np.float64) for s in range(2)]
        spos = [0, 0]
        for g in range(NG):
            for t in range(max(plan.tc[g])):
                for s in range(2):
                    if t >= plan.tc[g][s]:
                        continue
                    ss, dd, vv = buckets[(g, s)]
                    ss = ss[t * CAP:(t + 1) * CAP]
                    dd = dd[t * CAP:(t + 1) * CAP]
                    vv = vv[t * CAP:(t + 1) * CAP]
                    pos = spos[s]
                    base = plan.db[g][s][t]
                    idx_lin[s][pos * CAP:pos * CAP + len(ss)] = ss
                    np.add.at(sval[s],
                              (np.arange(len(ss)), pos * bw + dd - base), vv)
                    spos[s] += 1
        in_maps.append({
            "xplo": xplo, "xphi": xphi,
            "gilo": np.ascontiguousarray(wrap_idx(idx_lin[0])),
            "gihi": np.ascontiguousarray(wrap_idx(idx_lin[1])),
            "svlo": np.ascontiguousarray(
                sval[0].astype(np.float32).astype(
                    np.asarray(xb).dtype)),
            "svhi": np.ascontiguousarray(
                sval[1].astype(np.float32).astype(
                    np.asarray(xb).dtype)),
            "cst": np.ascontiguousarray(
                np.concatenate([bb, wt], axis=1), dtype=np.float32),
        })
    return plan, in_maps


def _build_nc(cfg: Cfg, plan: Plan):
    from contextlib import ExitStack

    import concourse.bacc as bacc
    import concourse.mybir as mybir
    import concourse.tile as tile

    f32 = mybir.dt.float32
    bf16 = mybir.dt.bfloat16
    i16 = mybir.dt.int16
    NG = cfg.n_groups
    RPC = cfg.rows_per_core
    CH = cfg.chunk
    SPLIT = cfg.split
    BW = plan.bw
    tp = [math.ceil(plan.t_stream(s) / CH) * CH for s in range(2)]

    CW = 2 * D
    O_BB, O_WT = 0, D

    nc = bacc.Bacc("TRN2", target_bir_lowering=False,
                   num_swdge_queues=cfg.gather_queues,
                   dynamic_dma_scratch_size=cfg.dma_scratch)
    xp = [nc.dram_tensor("xplo", [SPLIT, D], bf16, kind="ExternalInput"),
          nc.dram_tensor("xphi", [SPLIT, D], bf16, kind="ExternalInput")]
    gi = [nc.dram_tensor("gilo", [128, tp[0] * 8], i16, kind="ExternalInput"),
          nc.dram_tensor("gihi", [128, tp[1] * 8], i16, kind="ExternalInput")]
    sv = [nc.dram_tensor("svlo", [128, tp[0] * BW], bf16,
                         kind="ExternalInput"),
          nc.dram_tensor("svhi", [128, tp[1] * BW], bf16,
                         kind="ExternalInput")]
    cst = nc.dram_tensor("cst", [128, CW], f32, kind="ExternalInput")
    y = nc.dram_tensor("y", [NG * GR, D], f32, kind="ExternalOutput")

    with tile.TileContext(nc) as tc, ExitStack() as ctx:
        const = ctx.enter_context(tc.tile_pool(name="const", bufs=1))
        gpool = [ctx.enter_context(tc.tile_pool(name="glo",
                                                bufs=cfg.gather_bufs)),
                 ctx.enter_context(tc.tile_pool(name="ghi",
                                                bufs=cfg.gather_bufs))]
        epool = ctx.enter_context(tc.tile_pool(name="epilog", bufs=2))
        ps_agg = ctx.enter_context(tc.tile_pool(name="psagg", bufs=cfg.agg_bufs,
                                                space="PSUM"))
        ps_out = ctx.enter_context(tc.tile_pool(name="psout", bufs=2,
                                                space="PSUM"))

        cst_sb = const.tile([128, CW], f32)
        nc.sync.dma_start(out=cst_sb[:], in_=cst[:])

        def bb_ap(p):
            return cst_sb[:p, O_BB:O_BB + D]

        def wt_ap():
            return cst_sb[:, O_WT:O_WT + D]

        gi_sb = []
        sv_sb = []
        for s in range(2):
            t = const.tile([128, tp[s] * 8], i16, tag=f"gi{s}")
            nc.sync.dma_start(out=t[:], in_=gi[s][:])
            gi_sb.append(t)
            t = const.tile([128, tp[s] * BW], bf16, tag=f"sv{s}")
            nc.sync.dma_start(out=t[:], in_=sv[s][:])
            sv_sb.append(t)

        gbuf = [None, None]

        def fetch_chunk(s, ci):
            ge = cfg.gelem
            # trim the final call to the tiles actually consumed
            n = min(CH, plan.t_stream(s) - ci * CH)
            gbuf[s] = gpool[s].tile([128, CH * max(D, ge)], bf16, tag=f"gb{s}",
                                    name=f"gbuf{s}")
            nc.gpsimd.dma_gather(
                gbuf[s][:, :n * ge].rearrange("p (k j) -> p k j", j=ge),
                xp[s][:] if ge == D
                else (xp[s].rearrange("a (c d) -> (a c) d", d=ge) if ge < D
                      else xp[s].rearrange("(a c) d -> a (c d)", c=ge // D)),
                gi_sb[s][:, ci * CH * 8:ci * CH * 8 + n * 8],
                n * 128, n * 128, ge,
                single_packet=(CH * 128 <= 1024),
                queue_num=(s * 2 + ci % 2 if cfg.gather_queues == 4
                           else s % cfg.gather_queues),
            )

        def body():
            if cfg.skip_compute:
                for ci in range(max(tp) // CH):
                    for s in range(2):
                        if ci < tp[s] // CH:
                            fetch_chunk(s, ci)
                return
            spos = [0, 0]
            for g in range(NG):
                rows_g = min(GR, RPC - g * GR)
                agg = ps_agg.tile([128, GR], f32)
                first = True
                for t in range(max(plan.tc[g])):
                    for s in range(2):
                        if t >= plan.tc[g][s]:
                            continue
                        if spos[s] % CH == 0:
                            fetch_chunk(s, spos[s] // CH)
                        k = spos[s] % CH
                        base = plan.db[g][s][t]
                        nc.tensor.matmul(
                            out=agg[:, base:base + BW],
                            lhsT=gbuf[s][:, k * D:(k + 1) * D],
                            rhs=sv_sb[s][:, spos[s] * BW:(spos[s] + 1) * BW],
                            start=first, stop=True,
                            skip_group_check=True,
                        )
                        first = False
                        spos[s] += 1
                agg_sb = epool.tile([128, GR], f32, tag="aggsb")
                nc.vector.tensor_copy(out=agg_sb[:, :rows_g],
                                      in_=agg[:, :rows_g])
                for rc in range(math.ceil(rows_g / 128)):
                    w = min(128, rows_g - rc * 128)
                    out_ps = ps_out.tile([128, D], f32)
                    nc.tensor.matmul(
                        out=out_ps[:w, :],
                        lhsT=agg_sb[:, rc * 128:rc * 128 + w],
                        rhs=wt_ap(),
                        start=True, stop=True,
                    )
                    out_sb = epool.tile([128, D], f32, tag="outsb")
                    nc.vector.tensor_tensor(
                        out=out_sb[:w, :], in0=out_ps[:w, :], in1=bb_ap(w),
                        op=mybir.AluOpType.add,
                    )
                    r0 = g * GR + rc * 128
                    nc.sync.dma_start(out=y[r0:r0 + w, :], in_=out_sb[:w, :])

        if cfg.loop_n > 0:
            with tc.For_i(0, cfg.loop_n, 1):
                body()
        else:
            for _ in range(cfg.repeats):
                body()

    nc.compile()
    return nc


_CACHE = {}


def _get_nc(cfg: Cfg, plan: Plan):
    key = (cfg, plan)
    if key not in _CACHE:
        _CACHE[key] = _build_nc(cfg, plan)
    return _CACHE[key]


def kernel(x, edge_row, edge_col, edge_val, W, b):
    from concourse.bass_utils import run_bass_kernel_spmd

    cfg = Cfg()
    plan, in_maps = _preprocess(cfg, x, edge_row, edge_col, edge_val, W, b)
    nc = _get_nc(cfg, plan)
    res = run_bass_kernel_spmd(nc, in_maps, core_ids=list(range(cfg.n_cores)))
    RPC = cfg.rows_per_core
    out = np.empty((cfg.n_nodes, D), dtype=np.float32)
    for c in range(cfg.n_cores):
        out[c * RPC:(c + 1) * RPC] = res.results[c]["y"][:RPC]
    return out
